# revision 1
# baseline (speedup 1.0000x reference)
"""Bass/Trainium2 kernel for DepthAttentionResidual.

Math (per (b, t) position, S=16 sources, D=2048):
    ss[s]  = sum_d x[s]^2
    qx[s]  = sum_d q[d] * x[s, d]
    score  = qx * rsqrt(ss/D + eps) / sqrt(D)          # keys never materialized
    w      = softmax_s(score)                          # no max-subtract: |score| ~ N(0,1)
    out[d] = sum_s w[s] * x[s, d]

Sharding: data-parallel over (B x T/2) -> 8 cores; each core gets
x_c = source_bank[:, b, half] of shape [16, 512, 2048] (64 MB) and produces
[512, 2048].

On-chip layout: each SBUF x-tile is [128, 2048] with partition p = tl*4 + sq,
covering 32 t-rows (tl) x 4 sources (sq).  A "t-group" of 32 rows therefore
spans QT=4 such tiles (source quarters).  Per tile:
  - ScalarE: Square activation with accum_out  -> ss column
  - VectorE: tensor_tensor_reduce(x, q_bcast)  -> qx column
  - softmax over s: partition-group sums via a tiny TensorE matmul
    (indicator I32), a free-dim reduce over quarters, reciprocal, and a
    broadcast-back matmul (indicator J4)
  - weighted sum over s: TensorE matmul with lhsT = w * I32 (sparse weights),
    accumulating over the 4 quarters in PSUM.  A batch of 16 tiles =
    128 t-rows fills one [128, 2048] PSUM tile at 32-aligned offsets ->
    one PSUM->SBUF copy -> one 1 MB DMA out.
"""

import math

import numpy as np

S, B, T, D = 16, 4, 1024, 2048
N_CORES = 8
SQ = 4                    # sources per partition-group
QT = S // SQ              # 4 source-quarters
TL = 32                   # t-rows per x-tile
G = 4                     # t-groups per batch
BATCH = G * QT            # 16 x-tiles per batch -> 128 t-rows
P = 128
T_CORE = (B * T) // N_CORES          # 512 t-rows per core
EPS = float(np.finfo(np.float32).eps)
SCALE = 1.0 / math.sqrt(D)           # TEMPERATURE = 1.0
MM_F32R = True            # f32r matmuls: 1 cyc/row vs 4 for fp32; dst must be partition 0

_module_cache = {}


def build_module(t_core=T_CORE, x_bufs=5, mm_f32r=MM_F32R, groups_per_batch=G, reps=1):
    import concourse.bass as bass
    import concourse.bacc as bacc
    import concourse.mybir as mybir
    import concourse.tile as tile
    from concourse.dve_ops import TENSOR_TENSOR_REDUCE as TTR_OP

    fp32 = mybir.dt.float32
    f32r = mybir.dt.float32r
    gn = groups_per_batch
    batch = gn * QT
    rows_per_batch = TL * gn
    n_batches = t_core // rows_per_batch
    assert n_batches * rows_per_batch == t_core

    x_dt = f32r if mm_f32r else fp32
    nc = bacc.Bacc(None)
    x_h = nc.declare_dram_parameter("x", [S, t_core, D], x_dt, isOutput=False)
    q_h = nc.declare_dram_parameter("q", [D], fp32, isOutput=False)
    o_h = nc.declare_dram_parameter("out", [t_core, D], fp32, isOutput=True)

    # Indicator matrices for partition-group ops (partition p = tl*4 + sq).
    i32 = np.zeros((P, TL), np.float32)
    i32[np.arange(P), np.arange(P) // SQ] = 1.0        # group-sum over sq
    j4 = np.zeros((TL, P), np.float32)
    j4[np.arange(P) // SQ, np.arange(P)] = 1.0         # broadcast back per group
    i32_h = nc.inline_tensor(i32, name="i32const")
    j4_h = nc.inline_tensor(j4, name="j4const")

    x_ap = x_h[:]
    q_ap = q_h[:]
    q_bcast = bass.AP(tensor=q_ap.tensor, offset=q_ap.offset, ap=[[0, P], *q_ap.ap])

    AF = mybir.ActivationFunctionType
    OP = mybir.AluOpType

    with tile.TileContext(nc) as tc:
        with (
            tc.tile_pool(name="xpool", bufs=x_bufs) as xpool,
            tc.tile_pool(name="single", bufs=1) as single,
            tc.tile_pool(name="stats", bufs=4) as stats,
            tc.tile_pool(name="wepool", bufs=4) as wepool,
            tc.tile_pool(name="opool", bufs=1) as opool,
            tc.tile_pool(name="ppool", bufs=4 if mm_f32r else 1, space="PSUM") as ppool,
            tc.tile_pool(name="pspool", bufs=1, space="PSUM") as pspool,
            tc.tile_pool(name="warmp", bufs=1, space="PSUM") as warmp,
        ):
            q_sb = single.tile([P, D], fp32)
            nc.sync.dma_start(out=q_sb, in_=q_bcast)
            i32_sb = single.tile([P, TL], fp32)
            nc.sync.dma_start(out=i32_sb, in_=i32_h[:])
            j4_sb = single.tile([TL, P], fp32)
            nc.sync.dma_start(out=j4_sb, in_=j4_h[:])
            eps_sb = single.tile([P, 1], fp32)
            nc.vector.memset(eps_sb, EPS)
            # discarded elementwise outputs (only accum_out is consumed);
            # bf16 halves their SBUF footprint
            g_act = single.tile([P, D], mybir.dt.bfloat16)
            g_dve = single.tile([P, D], mybir.dt.bfloat16)

            import contextlib
            rep_ctx = (
                tc.For_i(0, reps, 1) if reps > 1 else contextlib.nullcontext()
            )
            with rep_ctx:
                for bi in range(n_batches):
                    ss_col = stats.tile([P, batch], fp32, tag="ss")
                    qx_col = stats.tile([P, batch], fp32, tag="qx")
                    ostage = opool.tile([rows_per_batch, D], fp32, tag="os")
                    xt = []
                    for g in range(gn):
                        t0 = bi * rows_per_batch + g * TL
                        # 4 x 1MB DMAs per t-group into one [P, QT, D] tile
                        # (a single 4MB DMA needs a 4-D access pattern, which
                        # the DMA AP balancer rejects)
                        xs = xpool.tile([P, QT, D], x_dt, tag="x")
                        for qt in range(QT):
                            src = x_ap[
                                qt * SQ : (qt + 1) * SQ, t0 : t0 + TL, :
                            ].rearrange("s tl d -> tl s d")
                            nc.sync.dma_start(out=xs[:, qt, :], in_=src)
                        xt.append(xs)
                        xs_f = xs.bitcast(fp32) if mm_f32r else xs
                        for qt in range(QT):
                            j = g * QT + qt
                            nc.scalar.activation(
                                out=g_act, in_=xs_f[:, qt, :], func=AF.Square,
                                accum_out=ss_col[:, j : j + 1],
                            )
                            # ISA TENSOR_TENSOR_REDUCE crashes at runtime here;
                            # the custom-DVE ucode variant works.
                            nc.vector._custom_dve(
                                TTR_OP, out=g_dve, in0=xs_f[:, qt, :], in1=q_sb,
                                s0=0.0, s1=1.0,
                                accum_out=qx_col[:, j : j + 1],
                            )
                            # 1-row dummy matmul tied to this load keeps the
                            # PE clock-gate (HAM) warm between real bursts
                            wpo = warmp.tile([TL, 1], fp32, tag="wp")
                            nc.tensor.matmul(
                                wpo, i32_sb, xs_f[:, qt, 0:1],
                                start=True, stop=True,
                            )

                    # --- batch softmax over the 16 sources (per t-row) ---
                    # rsqrt(ms+eps) via Newton on DVE: avoids the Sqrt ACT
                    # table, so the only table set loaded is exp_and_friends
                    v = stats.tile([P, batch], fp32, tag="v")
                    nc.vector.tensor_scalar(
                        out=v, in0=ss_col, scalar1=1.0 / D, scalar2=EPS,
                        op0=OP.mult, op1=OP.add,
                    )
                    y = stats.tile([P, batch], fp32, tag="y")
                    nc.vector.tensor_scalar(
                        out=y, in0=v, scalar1=-0.5, scalar2=1.5,
                        op0=OP.mult, op1=OP.add,
                    )
                    for _ in range(2):
                        y2 = stats.tile([P, batch], fp32, tag="y2")
                        nc.vector.tensor_mul(y2, y, y)
                        vy2 = stats.tile([P, batch], fp32, tag="vy2")
                        nc.vector.tensor_mul(vy2, v, y2)
                        h = stats.tile([P, batch], fp32, tag="h")
                        nc.vector.tensor_scalar(
                            out=h, in0=vy2, scalar1=-0.5, scalar2=1.5,
                            op0=OP.mult, op1=OP.add,
                        )
                        yn = stats.tile([P, batch], fp32, tag="yn")
                        nc.vector.tensor_mul(yn, y, h)
                        y = yn

                    sc = stats.tile([P, batch], fp32, tag="sc")
                    nc.vector.tensor_mul(sc, qx_col, y)
                    u = stats.tile([P, batch], fp32, tag="u")
                    nc.scalar.activation(out=u, in_=sc, func=AF.Exp, scale=SCALE)

                    pd = pspool.tile([TL, batch], fp32, tag="pd")
                    nc.tensor.matmul(pd, i32_sb, u, start=True, stop=True)
                    dsum = stats.tile([TL, gn], fp32, tag="dsum")
                    nc.vector.tensor_reduce(
                        out=dsum,
                        in_=pd.rearrange("p (g qt) -> p g qt", qt=QT),
                        axis=mybir.AxisListType.X,
                        op=OP.add,
                    )
                    rd = stats.tile([TL, gn], fp32, tag="rd")
                    nc.vector.reciprocal(out=rd, in_=dsum)
                    pbc = pspool.tile([P, gn], fp32, tag="pbc")
                    nc.tensor.matmul(pbc, j4_sb, rd, start=True, stop=True)
                    bc_sb = stats.tile([P, gn], fp32, tag="bc")
                    nc.vector.tensor_copy(bc_sb, pbc)
                    wn = stats.tile([P, batch], fp32, tag="wn")
                    for g in range(gn):
                        nc.vector.tensor_scalar(
                            out=wn[:, g * QT : (g + 1) * QT],
                            in0=u[:, g * QT : (g + 1) * QT],
                            scalar1=bc_sb[:, g : g + 1],
                            scalar2=None,
                            op0=OP.mult,
                        )

                    # weighted sum on PE, PSUM-accumulated over quarters;
                    # per-group weights/copies/out-DMAs release x-slots early
                    for g in range(gn):
                        we_all = wepool.tile([P, QT, TL], fp32, tag="wea")
                        for qt in range(QT):
                            j = g * QT + qt
                            nc.vector.tensor_scalar_mul(
                                we_all[:, qt, :], i32_sb, wn[:, j : j + 1]
                            )
                        if mm_f32r:
                            we_r = wepool.tile([P, QT, TL], f32r, tag="wer")
                            nc.gpsimd.dma_start(out=we_r, in_=we_all)
                        else:
                            we_r = we_all
                        if mm_f32r:
                            for ci, c0 in enumerate(range(0, D, 512)):
                                po = ppool.tile([TL, 512], fp32, tag="po")
                                for qt in range(QT):
                                    nc.tensor.matmul(
                                        po, we_r[:, qt, :],
                                        xt[g][:, qt, c0 : c0 + 512],
                                        start=(qt == 0), stop=(qt == QT - 1),
                                    )
                                dst = ostage[g * TL : (g + 1) * TL, c0 : c0 + 512]
                                # 3:1 ACT:DVE split - DVE is the busier engine
                                if ci % 4 != 3:
                                    nc.scalar.copy(out=dst, in_=po)
                                else:
                                    nc.vector.tensor_copy(dst, po)
                        else:
                            po = ppool.tile([TL, D], fp32, tag="po")
                            for qt in range(QT):
                                for c0 in range(0, D, 512):
                                    nc.tensor.matmul(
                                        po[:, c0 : c0 + 512],
                                        we_r[:, qt, :],
                                        xt[g][:, qt, c0 : c0 + 512],
                                        start=(qt == 0), stop=(qt == QT - 1),
                                    )
                            nc.scalar.copy(
                                out=ostage[g * TL : (g + 1) * TL, :], in_=po
                            )
                        nc.scalar.dma_start(
                            out=o_h[
                                bi * rows_per_batch + g * TL :
                                bi * rows_per_batch + (g + 1) * TL, :
                            ],
                            in_=ostage[g * TL : (g + 1) * TL, :],
                        )

    nc.compile()
    return nc


def _get_module():
    key = (T_CORE, MM_F32R)
    if key not in _module_cache:
        _module_cache[key] = build_module()
    return _module_cache[key]


def _run(layer_query, source_bank, **spmd_kwargs):
    from concourse.bass_utils import run_bass_kernel_spmd

    q = np.ascontiguousarray(np.asarray(layer_query, dtype=np.float32))
    x = np.asarray(source_bank, dtype=np.float32)
    assert x.shape == (S, B, T, D)

    nc = _get_module()
    in_maps = []
    for c in range(N_CORES):
        b, h = c // 2, c % 2
        xc = np.ascontiguousarray(x[:, b, h * T_CORE : (h + 1) * T_CORE, :])
        in_maps.append({"x": xc, "q": q})

    res = run_bass_kernel_spmd(nc, in_maps, core_ids=list(range(N_CORES)), **spmd_kwargs)
    full = np.empty((B, T, D), dtype=np.float32)
    for c in range(N_CORES):
        b, h = c // 2, c % 2
        full[b, h * T_CORE : (h + 1) * T_CORE, :] = res.results[c]["out"]
    return full, res


def kernel(layer_query, source_bank, num_sources=None):
    full, _ = _run(layer_query, source_bank)
    return full

